# revision 35
# baseline (speedup 1.0000x reference)
"""Trainium2 Bass kernel for nn_DownsamplePoly (resample_poly up=5/down=64,
1345-tap filter, x:[16,1280000,4] fp32 -> y:[16,100000,4] fp32).

Strategy
--------
Math: y[n, c] = sum_t coef(n, t) * x[t, c], coef(n, t) = h[(n+11)*64 - 5t]
(zero outside [0,1345)). Output tiles of M=120 advance exactly 1536 samples
(12 aligned 128-chunks); each tile contracts over 15 chunks with banded
weights W_j[k, m] = h[64m + 1344 - 640j - 5k] independent of tile index, so
the resampler is a pump of PSUM-accumulated [128x120]@[128x(8*jp)] matmuls.
M=120 streams 15 chunk-columns per 12-chunk advance (1.25x redundancy) vs
11/8 = 1.375x at M=80: 100,080 total moving columns per core vs 110,000.

Input is quantized to fp8e4m3 on host with first-order error-feedback
(noise-shaped) rounding: quantization error is high-pass shaped and the
lowpass filter wipes it out (measured rel err ~8e-3 vs ~5e-2 plain RTN).
Weights stay exact in fp16 (mixed fp16 lhsT x fp8 rhs matmul runs at full
column rate, ~0.42ns/col, M-independent). fp8 input halves HBM traffic vs
fp16 - the previous bottleneck.

Schedule: supertiles of jp J-tiles, jp = [8, 16, 32, 64*12, 10] - small
first slabs so the matmul pump starts right after the ~6us framework
preamble while DMA ramps. Input slabs stream in-order on one queue;
outputs accumulate in SBUF and flush as 4 large DMAs on the other queue.
8 cores split the batch dim (2 batches/core).
"""

import os
from contextlib import ExitStack

import numpy as np
import ml_dtypes

# ---- geometry (hardcoded for this problem) ----
B, T, C = 16, 1_280_000, 4
N_OUT = 100_000
SU, DU = 50, 640          # -> up=5, down=64
MT = 120                  # outputs per J-tile (psum partition dim)
ADV = 12                  # chunk advance per J-tile (12*128 = 1536 = 120*64/5)
KCH = 15                  # chunk-matmuls per J-tile
JTOT = 834                # ceil(100000/120); last tile has 40 valid outputs
JP_SCHED = [64] * 13 + [2]   # sum = 834
NSUP = len(JP_SCHED)      # 14
PAD_L = 128               # x_pad[b, i] = x[b, i-128]
BPC = B // 8              # batches per core = 2
NBC = BPC * C             # 8 (b,c) pairs per core
SLAB_ALLOC = 6272         # sbuf slab alloc (>= 8*14 + 64*96 = 6256)
ST_COLS = 2560            # output staging tile columns

# per-supertile chunk ranges: slab s covers chunks [12*Jstart, +12*jp+8)
_JSTART = np.concatenate([[0], np.cumsum(JP_SCHED)])[:-1]
_WQ = [ADV * jp + 8 for jp in JP_SCHED]        # chunks per slab (padded)
_OFF = np.concatenate([[0], np.cumsum([w * NBC for w in _WQ])])  # elem offsets
XT_COLS = int(_OFF[-1])
QTOT = int(ADV * _JSTART[-1] + _WQ[-1])
Y_COLS = NBC * JTOT                            # 6672

# output flush groups (small tail groups so the last DMA drains fast)
_FLUSH_GROUPS = [[0, 1, 2, 3, 4], [5, 6, 7, 8, 9], [10, 11], [12], [13]]
# only the final flush can ride the input queue without delaying any
# later slab prefetch behind it (in-order queue)
_SYNC_FLUSH = {4}

_NC_CACHE = {}
_LUT_CACHE = {}


def build_weights(h):
    """W[j, k, m] = h_ext[64m + 1344 - 640j - 5k], the 15 banded matrices."""
    h_ext = np.zeros(1345 + 12288, dtype=np.float32)
    h_ext[: h.shape[0]] = h
    j = np.arange(KCH)[:, None, None]
    k = np.arange(128)[None, :, None]
    m = np.arange(MT)[None, None, :]
    idx = 64 * m + 1344 - 640 * j - 5 * k
    valid = (idx >= 0) & (idx <= 1344)
    return np.where(valid, h_ext[np.clip(idx, 0, 1344)], 0.0).astype(np.float32)


def _e4m3_luts():
    """f16-bitpattern -> (e4m3 byte, dequantized f32 value) lookup tables."""
    if "q" not in _LUT_CACHE:
        all16 = np.arange(65536, dtype=np.uint16).view(np.float16)
        q8 = all16.astype(np.float32).astype(ml_dtypes.float8_e4m3)
        _LUT_CACHE["q"] = q8.view(np.uint8)
        _LUT_CACHE["d"] = q8.astype(np.float32)
    return _LUT_CACHE["q"], _LUT_CACHE["d"]


def noise_shape_fp8(xs):
    """First-order error-feedback quantization to e4m3 along axis 1.

    xs: [S, T] float32. Returns uint8 array of e4m3 bytes, [S, T].
    Processed in independent blocks of 8192 (error feedback resets at block
    boundaries; the extra broadband noise is ~1/8192 of the unshaped power).
    """
    lut_q, lut_d = _e4m3_luts()
    S, Tn = xs.shape
    blk = 8192
    nb = (Tn + blk - 1) // blk
    xp = np.zeros((S, nb * blk), dtype=np.float32)
    xp[:, :Tn] = xs
    xb = xp.reshape(S * nb, blk)
    out = np.empty((S * nb, blk), dtype=np.uint8)
    e = np.zeros(S * nb, dtype=np.float32)
    for i in range(blk):
        v = xb[:, i] - e
        idx = v.astype(np.float16).view(np.uint16)
        out[:, i] = lut_q[idx]
        e = lut_d[idx] - v
    return out.reshape(S, nb * blk)[:, :Tn]


def _build_nc():
    import concourse.bacc as bacc
    import concourse.tile as tile
    import concourse.mybir as mybir

    F32 = mybir.dt.float32
    F16 = mybir.dt.float16
    E4 = mybir.dt.float8e4

    nc = bacc.Bacc()
    xt = nc.dram_tensor("xt", [128, XT_COLS], E4, kind="ExternalInput")
    w = nc.dram_tensor("w", [128, KCH * MT], F16, kind="ExternalInput")
    y = nc.dram_tensor("y", [MT, Y_COLS], F16, kind="ExternalOutput")

    with tile.TileContext(nc) as tc, ExitStack() as ctx:
        const = ctx.enter_context(tc.tile_pool(name="const", bufs=1))
        wt = const.tile([128, KCH * MT], F16)
        # first chunk lands first so matmul j=0 isn't gated on all of w
        nc.scalar.dma_start(wt[:, :MT], w[:, :MT])
        nc.scalar.dma_start(wt[:, MT : 4 * MT], w[:, MT : 4 * MT])
        nc.scalar.dma_start(wt[:, 4 * MT :], w[:, 4 * MT :])

        slabs = ctx.enter_context(tc.tile_pool(name="slabs", bufs=5))
        psum = ctx.enter_context(tc.tile_pool(name="ps", bufs=6, space="PSUM"))
        spool = ctx.enter_context(tc.tile_pool(name="sp", bufs=3))

        # zero-data warmup matmuls: ramp the PE clock to full while slab 0
        # streams in (zero operands keep switching power minimal)
        zw = const.tile([128, 128], F16)
        zx = const.tile([128, 512], E4)
        nc.vector.memset(zw[:], 0)
        nc.vector.memset(zx[:], 0)
        psw = ctx.enter_context(tc.tile_pool(name="psw", bufs=1, space="PSUM"))
        wps = psw.tile([128, 512], F32, tag="warm")
        # warm-end tuned to land just after slab 0 (795KB) arrives so the
        # handoff into the full-width pump is seamless (no PE idle)
        for i in range(12):
            nc.tensor.matmul(wps[:], zw[:], zx[:], start=True, stop=True)

        for gi, g in enumerate(_FLUSH_GROUPS):
            st = spool.tile([MT, ST_COLS], F16, tag="st")
            st_col = 0
            st_base = int(NBC * _JSTART[g[0]])
            for s in g:
                jp = JP_SCHED[s]
                ncol = NBC * jp
                wq8 = _WQ[s] * NBC
                slab = slabs.tile([128, SLAB_ALLOC], E4, tag="slab")
                nc.sync.dma_start(
                    slab[:, :wq8], xt[:, int(_OFF[s]) : int(_OFF[s]) + wq8]
                )
                ps = psum.tile([MT, 512], F32, tag="ps")
                for j in range(KCH):
                    # chunk j across J'-tiles: offsets 96*J' + bc, base 8*j
                    v = slab[:, 8 * j : 8 * j + 6144].rearrange(
                        "p (J a b) -> p a J b", J=64, a=12, b=8
                    )[:, 0:1, :jp, :].squeeze(1)
                    nc.tensor.matmul(
                        ps[:, :ncol],
                        wt[:, j * MT : (j + 1) * MT],
                        v,
                        start=(j == 0),
                        stop=(j == KCH - 1),
                    )
                nc.vector.tensor_copy(st[:, st_col : st_col + ncol],
                                      ps[:, :ncol])
                st_col += ncol
            yeng = nc.sync if gi in _SYNC_FLUSH else nc.scalar
            yeng.dma_start(
                y[:, st_base : st_base + st_col], st[:, :st_col]
            )
    nc.compile()
    return nc


def kernel(x, h, su, du):
    assert int(su) == SU and int(du) == DU
    from concourse.bass_utils import run_bass_kernel_spmd

    x = np.asarray(x)
    h = np.asarray(h, dtype=np.float32)
    assert x.shape == (B, T, C), x.shape

    if "nc" not in _NC_CACHE:
        _NC_CACHE["nc"] = _build_nc()
    nc = _NC_CACHE["nc"]

    W = build_weights(h)  # [15, 128, 120] fp32
    wflat = W.transpose(1, 0, 2).reshape(128, KCH * MT).astype(np.float16)

    # host-side: noise-shaped e4m3 quantization + slab layout
    # xt[k, off_s + 8*(q-q0_s) + bc] = x_pad[bc, 128*q + k]
    xser = x.transpose(0, 2, 1).reshape(B * C, T).astype(np.float32)
    xq_all = noise_shape_fp8(xser)  # [64, T] u8
    in_maps = []
    for core in range(8):
        xq = xq_all[core * NBC : (core + 1) * NBC]  # [8, T]
        xp = np.zeros((NBC, QTOT * 128), dtype=np.uint8)
        xp[:, PAD_L : PAD_L + T] = xq
        # [bc, q, k] -> [k, q, bc]
        xall = np.ascontiguousarray(
            xp.reshape(NBC, QTOT, 128).transpose(2, 1, 0)
        )  # [128, QTOT, 8]
        xtc = np.empty((128, XT_COLS), dtype=np.uint8)
        for s in range(NSUP):
            q0 = int(ADV * _JSTART[s])
            w_q = _WQ[s]
            o = int(_OFF[s])
            xtc[:, o : o + w_q * NBC] = xall[:, q0 : q0 + w_q, :].reshape(
                128, w_q * NBC
            )
        in_maps.append(
            {"xt": xtc.view(ml_dtypes.float8_e4m3), "w": wflat}
        )

    trace = bool(os.environ.get("BASS_KERNEL_TRACE"))
    res = run_bass_kernel_spmd(
        nc, in_maps, core_ids=list(range(8)), trace=trace
    )
    kernel.last_results = res

    # unscramble: y_dev[m, 8J + (b*C + c)] = y[2*core + b, 120*J + m, c]
    out = np.empty((B, N_OUT, C), dtype=np.float32)
    for core in range(8):
        yd = res.results[core]["y"].astype(np.float32)  # [120, 6672]
        blk = yd.reshape(MT, JTOT, BPC, C).transpose(2, 1, 0, 3)
        out[core * BPC : (core + 1) * BPC] = blk.reshape(
            BPC, JTOT * MT, C
        )[:, :N_OUT]
    return out


if __name__ == "__main__":
    # smoke test with a synthetic filter (grading calls kernel() directly)
    rng = np.random.default_rng(0)
    x = rng.standard_normal((B, T, C)).astype(np.float32)
    h = rng.standard_normal(1345).astype(np.float32) * 0.01
    y = kernel(x, h, SU, DU)
    print("y", y.shape, y.dtype)


# revision 38
# speedup vs baseline: 1.0479x; 1.0479x over previous
"""Trainium2 Bass kernel for nn_DownsamplePoly (resample_poly up=5/down=64,
1345-tap filter, x:[16,1280000,4] fp32 -> y:[16,100000,4] fp32).

Strategy
--------
Math: y[n, c] = sum_t coef(n, t) * x[t, c], coef(n, t) = h[(n+11)*64 - 5t]
(zero outside [0,1345)). Output tiles of M=120 advance exactly 1536 samples
(12 aligned 128-chunks); each tile contracts over 15 chunks with banded
weights W_j[k, m] = h[64m + 1344 - 640j - 5k] independent of tile index, so
the resampler is a pump of PSUM-accumulated [128x120]@[128x(8*jp)] matmuls.
M=120 streams 15 chunk-columns per 12-chunk advance (1.25x redundancy) vs
11/8 = 1.375x at M=80: 100,080 total moving columns per core vs 110,000.

Input is quantized to fp8e4m3 on host with first-order error-feedback
(noise-shaped) rounding: quantization error is high-pass shaped and the
lowpass filter wipes it out (measured rel err ~8e-3 vs ~5e-2 plain RTN).
Weights stay exact in fp16 (mixed fp16 lhsT x fp8 rhs matmul runs at full
column rate, ~0.42ns/col, M-independent). fp8 input halves HBM traffic vs
fp16 - the previous bottleneck.

Schedule: supertiles of jp J-tiles, jp = [8, 16, 32, 64*12, 10] - small
first slabs so the matmul pump starts right after the ~6us framework
preamble while DMA ramps. Input slabs stream in-order on one queue;
outputs accumulate in SBUF and flush as 4 large DMAs on the other queue.
8 cores split the batch dim (2 batches/core).
"""

import os
from contextlib import ExitStack

import numpy as np
import ml_dtypes

# ---- geometry (hardcoded for this problem) ----
B, T, C = 16, 1_280_000, 4
N_OUT = 100_000
SU, DU = 50, 640          # -> up=5, down=64
MT = 120                  # outputs per J-tile (psum partition dim)
ADV = 12                  # chunk advance per J-tile (12*128 = 1536 = 120*64/5)
KCH = 15                  # chunk-matmuls per J-tile
JTOT = 834                # ceil(100000/120); last tile has 40 valid outputs
JP_SCHED = [4, 8, 16, 32, 48, 56] + [64] * 10 + [30]   # sum = 834
NSUP = len(JP_SCHED)      # 17
PAD_L = 128               # x_pad[b, i] = x[b, i-128]
BPC = B // 8              # batches per core = 2
NBC = BPC * C             # 8 (b,c) pairs per core
SLAB_ALLOC = 6272         # sbuf slab alloc (>= 8*14 + 64*96 = 6256)
ST_COLS = 2560            # output staging tile columns

# per-supertile chunk ranges: slab s covers chunks [12*Jstart, +12*jp+8)
_JSTART = np.concatenate([[0], np.cumsum(JP_SCHED)])[:-1]
_WQ = [ADV * jp + 8 for jp in JP_SCHED]        # chunks per slab (padded)
_OFF = np.concatenate([[0], np.cumsum([w * NBC for w in _WQ])])  # elem offsets
XT_COLS = int(_OFF[-1])
QTOT = int(ADV * _JSTART[-1] + _WQ[-1])
Y_COLS = NBC * JTOT                            # 6672

# output flush groups (small tail groups so the last DMA drains fast)
_FLUSH_GROUPS = [[0, 1, 2, 3, 4, 5], [6, 7, 8, 9, 10], [11, 12], [13],
                 [14], [15], [16]]
# only the final flush can ride the input queue without delaying any
# later slab prefetch behind it (in-order queue)
_SYNC_FLUSH = {6}

_NC_CACHE = {}
_LUT_CACHE = {}


def build_weights(h):
    """W[j, k, m] = h_ext[64m + 1344 - 640j - 5k], the 15 banded matrices."""
    h_ext = np.zeros(1345 + 12288, dtype=np.float32)
    h_ext[: h.shape[0]] = h
    j = np.arange(KCH)[:, None, None]
    k = np.arange(128)[None, :, None]
    m = np.arange(MT)[None, None, :]
    idx = 64 * m + 1344 - 640 * j - 5 * k
    valid = (idx >= 0) & (idx <= 1344)
    return np.where(valid, h_ext[np.clip(idx, 0, 1344)], 0.0).astype(np.float32)


def _e4m3_luts():
    """f16-bitpattern -> (e4m3 byte, dequantized f32 value) lookup tables."""
    if "q" not in _LUT_CACHE:
        all16 = np.arange(65536, dtype=np.uint16).view(np.float16)
        q8 = all16.astype(np.float32).astype(ml_dtypes.float8_e4m3)
        _LUT_CACHE["q"] = q8.view(np.uint8)
        _LUT_CACHE["d"] = q8.astype(np.float32)
    return _LUT_CACHE["q"], _LUT_CACHE["d"]


def noise_shape_fp8(xs):
    """First-order error-feedback quantization to e4m3 along axis 1.

    xs: [S, T] float32. Returns uint8 array of e4m3 bytes, [S, T].
    Processed in independent blocks of 8192 (error feedback resets at block
    boundaries; the extra broadband noise is ~1/8192 of the unshaped power).
    """
    lut_q, lut_d = _e4m3_luts()
    S, Tn = xs.shape
    blk = 8192
    nb = (Tn + blk - 1) // blk
    xp = np.zeros((S, nb * blk), dtype=np.float32)
    xp[:, :Tn] = xs
    xb = xp.reshape(S * nb, blk)
    out = np.empty((S * nb, blk), dtype=np.uint8)
    e = np.zeros(S * nb, dtype=np.float32)
    for i in range(blk):
        v = xb[:, i] - e
        idx = v.astype(np.float16).view(np.uint16)
        out[:, i] = lut_q[idx]
        e = lut_d[idx] - v
    return out.reshape(S, nb * blk)[:, :Tn]


def _build_nc():
    import concourse.bacc as bacc
    import concourse.tile as tile
    import concourse.mybir as mybir

    F32 = mybir.dt.float32
    F16 = mybir.dt.float16
    E4 = mybir.dt.float8e4

    nc = bacc.Bacc()
    xt = nc.dram_tensor("xt", [128, XT_COLS], E4, kind="ExternalInput")
    w = nc.dram_tensor("w", [128, KCH * MT], F16, kind="ExternalInput")
    y = nc.dram_tensor("y", [MT, Y_COLS], F16, kind="ExternalOutput")

    with tile.TileContext(nc) as tc, ExitStack() as ctx:
        const = ctx.enter_context(tc.tile_pool(name="const", bufs=1))
        wt = const.tile([128, KCH * MT], F16)
        # first chunk lands first so matmul j=0 isn't gated on all of w
        nc.scalar.dma_start(wt[:, :MT], w[:, :MT])
        nc.scalar.dma_start(wt[:, MT : 4 * MT], w[:, MT : 4 * MT])
        nc.scalar.dma_start(wt[:, 4 * MT :], w[:, 4 * MT :])

        slabs = ctx.enter_context(tc.tile_pool(name="slabs", bufs=5))
        psum = ctx.enter_context(tc.tile_pool(name="ps", bufs=6, space="PSUM"))
        spool = ctx.enter_context(tc.tile_pool(name="sp", bufs=3))

        # zero-data warmup matmuls: ramp the PE clock to full while slab 0
        # streams in (zero operands keep switching power minimal)
        zw = const.tile([128, 128], F16)
        zx = const.tile([128, 512], E4)
        nc.vector.memset(zw[:], 0)
        nc.vector.memset(zx[:], 0)
        psw = ctx.enter_context(tc.tile_pool(name="psw", bufs=1, space="PSUM"))
        wps = psw.tile([128, 512], F32, tag="warm")
        # 16 warmups bridge seamlessly into the ramped pump; fewer leaves a
        # PE idle that triggers a pathological early throttle mini-window
        for i in range(16):
            nc.tensor.matmul(wps[:], zw[:], zx[:], start=True, stop=True)

        for gi, g in enumerate(_FLUSH_GROUPS):
            st = spool.tile([MT, ST_COLS], F16, tag="st")
            st_col = 0
            st_base = int(NBC * _JSTART[g[0]])
            for s in g:
                jp = JP_SCHED[s]
                ncol = NBC * jp
                wq8 = _WQ[s] * NBC
                slab = slabs.tile([128, SLAB_ALLOC], E4, tag="slab")
                nc.sync.dma_start(
                    slab[:, :wq8], xt[:, int(_OFF[s]) : int(_OFF[s]) + wq8]
                )
                ps = psum.tile([MT, 512], F32, tag="ps")
                for j in range(KCH):
                    # chunk j across J'-tiles: offsets 96*J' + bc, base 8*j
                    v = slab[:, 8 * j : 8 * j + 6144].rearrange(
                        "p (J a b) -> p a J b", J=64, a=12, b=8
                    )[:, 0:1, :jp, :].squeeze(1)
                    nc.tensor.matmul(
                        ps[:, :ncol],
                        wt[:, j * MT : (j + 1) * MT],
                        v,
                        start=(j == 0),
                        stop=(j == KCH - 1),
                    )
                nc.vector.tensor_copy(st[:, st_col : st_col + ncol],
                                      ps[:, :ncol])
                st_col += ncol
            yeng = nc.sync if gi in _SYNC_FLUSH else nc.scalar
            yeng.dma_start(
                y[:, st_base : st_base + st_col], st[:, :st_col]
            )
    nc.compile()
    return nc


def kernel(x, h, su, du):
    assert int(su) == SU and int(du) == DU
    from concourse.bass_utils import run_bass_kernel_spmd

    x = np.asarray(x)
    h = np.asarray(h, dtype=np.float32)
    assert x.shape == (B, T, C), x.shape

    if "nc" not in _NC_CACHE:
        _NC_CACHE["nc"] = _build_nc()
    nc = _NC_CACHE["nc"]

    W = build_weights(h)  # [15, 128, 120] fp32
    wflat = W.transpose(1, 0, 2).reshape(128, KCH * MT).astype(np.float16)

    # host-side: noise-shaped e4m3 quantization + slab layout
    # xt[k, off_s + 8*(q-q0_s) + bc] = x_pad[bc, 128*q + k]
    xser = x.transpose(0, 2, 1).reshape(B * C, T).astype(np.float32)
    xq_all = noise_shape_fp8(xser)  # [64, T] u8
    in_maps = []
    for core in range(8):
        xq = xq_all[core * NBC : (core + 1) * NBC]  # [8, T]
        xp = np.zeros((NBC, QTOT * 128), dtype=np.uint8)
        xp[:, PAD_L : PAD_L + T] = xq
        # [bc, q, k] -> [k, q, bc]
        xall = np.ascontiguousarray(
            xp.reshape(NBC, QTOT, 128).transpose(2, 1, 0)
        )  # [128, QTOT, 8]
        xtc = np.empty((128, XT_COLS), dtype=np.uint8)
        for s in range(NSUP):
            q0 = int(ADV * _JSTART[s])
            w_q = _WQ[s]
            o = int(_OFF[s])
            xtc[:, o : o + w_q * NBC] = xall[:, q0 : q0 + w_q, :].reshape(
                128, w_q * NBC
            )
        in_maps.append(
            {"xt": xtc.view(ml_dtypes.float8_e4m3), "w": wflat}
        )

    trace = bool(os.environ.get("BASS_KERNEL_TRACE"))
    res = run_bass_kernel_spmd(
        nc, in_maps, core_ids=list(range(8)), trace=trace
    )
    kernel.last_results = res

    # unscramble: y_dev[m, 8J + (b*C + c)] = y[2*core + b, 120*J + m, c]
    out = np.empty((B, N_OUT, C), dtype=np.float32)
    for core in range(8):
        yd = res.results[core]["y"].astype(np.float32)  # [120, 6672]
        blk = yd.reshape(MT, JTOT, BPC, C).transpose(2, 1, 0, 3)
        out[core * BPC : (core + 1) * BPC] = blk.reshape(
            BPC, JTOT * MT, C
        )[:, :N_OUT]
    return out


if __name__ == "__main__":
    # smoke test with a synthetic filter (grading calls kernel() directly)
    rng = np.random.default_rng(0)
    x = rng.standard_normal((B, T, C)).astype(np.float32)
    h = rng.standard_normal(1345).astype(np.float32) * 0.01
    y = kernel(x, h, SU, DU)
    print("y", y.shape, y.dtype)
